# revision 9
# baseline (speedup 1.0000x reference)
"""Multi-head GAT layer on 8 Trainium2 NeuronCores.

Edge-major streaming design (zero gather):
  - Nodes are ranked by in-degree (desc) and dealt round-robin to the 8
    cores, so every core's destination tile t draws from the same global
    rank block and shares one max-degree K_t -> one SPMD program, zero
    cross-core chunk padding.
  - Destination slot = partition; chunk k of tile t holds the k-th
    in-neighbor of each of the tile's 128 dsts (chunk 0 = self loop).
    The scatter matrix is therefore the IDENTITY for every chunk and the
    per-edge s_dst needs no gather: it is sdst[tile] at the same slot.
  - The host streams x columns in edge order (duplicated per edge) as
    fp16.  The device projects each 128-edge chunk with ONE matmul
    against Wp = [W^T | W^T a_j | W^T a_i] (264 cols), so each psum row
    is [xh(256) | s_src(4) | s_dst(4)].  No DRAM feature table, no
    dma_gather (descriptor generation on GPSIMD was the baseline's
    bottleneck: 939us), no one-hot P/PT streams.
  - Pad slots get a crafted x_pad column solving
    [W^T a_j | W^T a_i]^T x = -300, so their scores are ~-600, their
    exp-weight flushes to exactly 0 in fp16, and they contribute nothing
    to numerator or denominator.
  - Per tile: eg = s_dst + s_src; leaky_relu; w = exp(eg) (no max
    subtraction needed: real scores are O(1)); G *= w; psum accumulation
    of the K identity matmuls; den = reduce(w); out = num/den.
"""

import sys

sys.path.insert(0, "/opt/trn_rl_repo")

import numpy as np
import ml_dtypes

import concourse.bass as bass
import concourse.mybir as mybir
from concourse import bacc, tile
from concourse.bass_utils import run_bass_kernel_spmd

# Problem constants (hardcoded per contest rules).
N_NODES = 50000
CIN = 128
COUT = 64
H = 4
HC = H * COUT  # 256
NCORES = 8
TILE = 128
NLOC = N_NODES // NCORES  # 6250
T = (NLOC + TILE - 1) // TILE  # 49
NEG_SLOPE = 0.2
B_PAD = 300.0

F32 = mybir.dt.float32
FP16 = mybir.dt.float16

YGRP = 7  # tiles per output-write batch


class Plan:
    def __init__(self, K, KOFF, TC, src_ids, node_ids):
        self.K = K  # [T] chunks per tile
        self.KOFF = KOFF  # [T+1] chunk offsets
        self.TC = TC  # total chunks
        self.src_ids = src_ids  # [NCORES][TC*128] int32, -1 = pad slot
        self.node_ids = node_ids  # [NCORES][NLOC] node id per dst position


def _preprocess(edge_index):
    """Integer-only host preprocessing: slot layout per core."""
    src = edge_index[0].astype(np.int64)
    dst = edge_index[1].astype(np.int64)
    E = src.size

    deg = np.bincount(dst, minlength=N_NODES) + 1  # incl self loop
    order = np.argsort(-deg, kind="stable")  # rank -> node
    rank = np.empty(N_NODES, np.int64)
    rank[order] = np.arange(N_NODES)
    deg_sorted = deg[order]

    # K_t = max degree in rank block [t*1024, (t+1)*1024) = first entry.
    K = [int(deg_sorted[t * TILE * NCORES]) for t in range(T)]
    KOFF = np.concatenate([[0], np.cumsum(K)]).astype(np.int64)
    TC = int(KOFF[-1])
    NSLOT = TC * 128

    r = rank[dst]
    core = r % NCORES
    pos = r // NCORES
    tl = pos // TILE
    sl = pos % TILE

    # k-index: 1 + occurrence count among this dst's edges (self = 0).
    ordidx = np.argsort(r, kind="stable")
    sorted_r = r[ordidx]
    first = np.searchsorted(sorted_r, sorted_r, side="left")
    kk = np.empty(E, np.int64)
    kk[ordidx] = np.arange(E) - first + 1

    col = (KOFF[tl] + kk) * 128 + sl

    src_ids = np.full((NCORES, NSLOT), -1, np.int32)
    node_ids = []
    for c in range(NCORES):
        m = core == c
        src_ids[c][col[m]] = src[m]
        nid = order[c::NCORES]
        node_ids.append(nid)
        # self loops
        p = rank[nid] // NCORES
        cs = KOFF[p // TILE] * 128 + (p % TILE)
        src_ids[c][cs] = nid
    return Plan(K, KOFF, TC, src_ids, node_ids)


def _build_program(plan):
    nc = bacc.Bacc(None, target_bir_lowering=False)
    K = plan.K
    KOFF = plan.KOFF
    TC = plan.TC
    KMAX = max(K)

    xE_in = nc.dram_tensor("xE", [128, TC * 128], FP16, kind="ExternalInput")
    wp_in = nc.dram_tensor("Wp", [128, HC + 8], FP16, kind="ExternalInput")
    id_in = nc.dram_tensor("ident", [128, 128], FP16, kind="ExternalInput")
    y_out = nc.dram_tensor("y", [T * 128, HC], F32, kind="ExternalOutput")

    with tile.TileContext(nc) as tc:
        with (
            tc.tile_pool(name="persist", bufs=1) as pp,
            tc.tile_pool(name="xe", bufs=3) as xe,
            tc.tile_pool(name="gg", bufs=2) as gg,
            tc.tile_pool(name="wx", bufs=2) as wx,
            tc.tile_pool(name="egp", bufs=2) as egp,
            tc.tile_pool(name="psA", bufs=4, space="PSUM") as psA,
            tc.tile_pool(name="pso", bufs=3, space="PSUM") as pso,
            tc.tile_pool(name="fin", bufs=2) as fin,
            tc.tile_pool(name="mk", bufs=4) as mk,
        ):
            wp_sb = pp.tile([128, HC + 8], FP16)
            nc.sync.dma_start(wp_sb[:], wp_in[:])
            id_sb = pp.tile([128, 128], FP16)
            nc.sync.dma_start(id_sb[:], id_in[:])
            sdst = pp.tile([128, T, 4], FP16)

            y_v = y_out.rearrange("(t p) f -> p t f", p=128)
            state = {"yow": None, "y0": 0, "ny": 0}

            def _back(t, Kt, Gt):
                eg = egp.tile([128, KMAX, 4], FP16, tag="eg")
                nc.vector.tensor_add(
                    eg[:, 0:Kt, :],
                    Gt[:, 0:Kt, HC : HC + 4],
                    sdst[:, t, :].unsqueeze(1).broadcast_to([128, Kt, 4]),
                )
                # leaky_relu: (e*0.2) max e
                nc.vector.scalar_tensor_tensor(
                    eg[:, 0:Kt, :],
                    eg[:, 0:Kt, :],
                    NEG_SLOPE,
                    eg[:, 0:Kt, :],
                    mybir.AluOpType.mult,
                    mybir.AluOpType.max,
                )
                w = wx.tile([128, KMAX, 4], FP16, tag="w")
                nc.scalar.activation(
                    w[:, 0:Kt, :],
                    eg[:, 0:Kt, :],
                    mybir.ActivationFunctionType.Exp,
                )
                # xh columns are (o, h)-interleaved so the per-head weight
                # broadcast has a contiguous fp16 inner dim (DVE 2x mode).
                nc.vector.tensor_mul(
                    Gt[:, 0:Kt, 0:HC].rearrange("p c (o h) -> p c o h", h=H),
                    Gt[:, 0:Kt, 0:HC].rearrange("p c (o h) -> p c o h", h=H),
                    w[:, 0:Kt, :]
                    .unsqueeze(2)
                    .broadcast_to([128, Kt, COUT, H]),
                )
                po = pso.tile([128, HC], F32, tag="po")
                for k in range(Kt):
                    nc.tensor.matmul(
                        po[:],
                        id_sb[:],
                        Gt[:, k, 0:HC],
                        start=(k == 0),
                        stop=(k == Kt - 1),
                        skip_group_check=True,
                    )
                dn = mk.tile([128, 4], F32, tag="dn")
                nc.vector.tensor_reduce(
                    dn.unsqueeze(-1),
                    w[:, 0:Kt, :].rearrange("p k h -> p h k"),
                    mybir.AxisListType.X,
                    mybir.AluOpType.add,
                )
                rec = mk.tile([128, 4], F32, tag="rec")
                nc.vector.tensor_scalar(
                    dn[:], dn[:], 1e-30, None, mybir.AluOpType.add
                )
                nc.vector.reciprocal(rec[:], dn[:])
                if state["ny"] == 0:
                    state["yow"] = fin.tile(
                        [128, YGRP, HC], F32, tag="yow", name="yow"
                    )
                    state["y0"] = t
                nc.vector.tensor_mul(
                    state["yow"][:, state["ny"], :].rearrange(
                        "p (o h) -> p o h", h=H
                    ),
                    po.rearrange("p (o h) -> p o h", h=H),
                    rec.unsqueeze(1).broadcast_to([128, COUT, H]),
                )
                state["ny"] += 1
                if state["ny"] == YGRP or t == T - 1:
                    nc.sync.dma_start(
                        y_v[:, state["y0"] : state["y0"] + state["ny"], :],
                        state["yow"][:, 0 : state["ny"], :],
                    )
                    state["ny"] = 0

            pend = None
            for t in range(T):
                Kt = K[t]
                xet = xe.tile([128, KMAX * 128], FP16, tag="xet")
                nc.sync.dma_start(
                    xet[:, 0 : Kt * 128],
                    xE_in[:, KOFF[t] * 128 : (KOFF[t] + Kt) * 128],
                )
                Gt = gg.tile([128, KMAX, HC + 4], FP16, tag="G")
                for k in range(Kt):
                    ps = psA.tile([128, HC + 8], F32, tag="psA")
                    nc.tensor.matmul(
                        ps[:],
                        xet[:, k * 128 : (k + 1) * 128],
                        wp_sb[:],
                        start=True,
                        stop=True,
                    )
                    # Drain split ~30/70 DVE/ACT to balance engine load
                    # (DVE also owns the alpha-multiply).
                    if k % 7 < 2:
                        nc.vector.tensor_copy(
                            Gt[:, k, 0 : HC + 4], ps[:, 0 : HC + 4]
                        )
                    else:
                        nc.scalar.copy(Gt[:, k, 0 : HC + 4], ps[:, 0 : HC + 4])
                    if k == 0:
                        nc.vector.tensor_copy(
                            sdst[:, t, :], ps[:, HC + 4 : HC + 8]
                        )
                if pend is not None:
                    _back(*pend)
                pend = (t, Kt, Gt)
            _back(*pend)

    nc.compile()
    return nc


def _make_in_maps(plan, x, W, a):
    # Wp = [W^T (256, (o,h)-interleaved) | W^T a_j (4) | W^T a_i (4)].
    Wt = np.ascontiguousarray(W.transpose(2, 1, 0).reshape(CIN, HC))
    a_i = a[:, :COUT]  # dst
    a_j = a[:, COUT:]  # src
    w_aj = np.einsum("hoc,ho->ch", W, a_j)  # [cin, H]
    w_ai = np.einsum("hoc,ho->ch", W, a_i)
    Wp = np.concatenate([Wt, w_aj, w_ai], axis=1).astype(np.float16)

    # x_pad: scores -B for all heads on both src and dst sides.
    A = np.concatenate([w_aj.T, w_ai.T], axis=0)  # [8, cin]
    xpad = np.linalg.lstsq(A, -B_PAD * np.ones(8), rcond=None)[0]

    XT = np.ascontiguousarray(x.T).astype(np.float16)  # [128, N]
    xpad16 = xpad.astype(np.float16)

    ident = np.eye(128, dtype=np.float16)

    in_maps = []
    for c in range(NCORES):
        ids = plan.src_ids[c]
        ids2 = np.where(ids >= 0, ids, 0)
        xe = XT[:, ids2]
        xe[:, ids < 0] = xpad16[:, None]
        in_maps.append(
            {
                "xE": np.ascontiguousarray(xe),
                "Wp": Wp,
                "ident": ident,
            }
        )
    return in_maps


_CACHE = {}


def _get_compiled(edge_key, edge_index):
    if edge_key not in _CACHE:
        plan = _preprocess(edge_index)
        nc = _build_program(plan)
        _CACHE[edge_key] = (plan, nc)
    return _CACHE[edge_key]


def kernel(x, edge_index, W, a, num_nodes, _trace=False):
    x = np.asarray(x)
    edge_index = np.asarray(edge_index)
    W = np.asarray(W)
    a = np.asarray(a)

    edge_key = hash(edge_index.tobytes())
    plan, nc = _get_compiled(edge_key, edge_index)
    in_maps = _make_in_maps(plan, x, W, a)

    kw = {}
    if _trace:
        kw = dict(trace=True)
    res = run_bass_kernel_spmd(nc, in_maps, core_ids=list(range(NCORES)), **kw)

    out = np.empty((N_NODES, HC), np.float32)
    for c in range(NCORES):
        y = res.results[c]["y"][:NLOC]
        # device columns are (o, h); reference wants (h, o)
        out[plan.node_ids[c]] = (
            y.reshape(NLOC, COUT, H).transpose(0, 2, 1).reshape(NLOC, HC)
        )
    if _trace:
        return out, res
    return out


# revision 12
# speedup vs baseline: 1.1595x; 1.1595x over previous
"""Multi-head GAT layer on 8 Trainium2 NeuronCores.

Edge-major streaming design (zero gather):
  - Nodes are ranked by in-degree (desc) and dealt round-robin to the 8
    cores, so every core's destination tile t draws from the same global
    rank block and shares one max-degree K_t -> one SPMD program, zero
    cross-core chunk padding.
  - Destination slot = partition; chunk k of tile t holds the k-th
    in-neighbor of each of the tile's 128 dsts (chunk 0 = self loop).
    The scatter matrix is therefore the IDENTITY for every chunk and the
    per-edge s_dst needs no gather: it is sdst[tile] at the same slot.
  - The host streams x columns in edge order (duplicated per edge) as
    fp16.  The device projects each 128-edge chunk with ONE matmul
    against Wp = [W^T | W^T a_j | W^T a_i] (264 cols), so each psum row
    is [xh(256) | s_src(4) | s_dst(4)].  No DRAM feature table, no
    dma_gather (descriptor generation on GPSIMD was the baseline's
    bottleneck: 939us), no one-hot P/PT streams.
  - Pad slots get a crafted x_pad column solving
    [W^T a_j | W^T a_i]^T x = -300, so their scores are ~-600, their
    exp-weight flushes to exactly 0 in fp16, and they contribute nothing
    to numerator or denominator.
  - Per tile: eg = s_dst + s_src; leaky_relu; w = exp(eg) (no max
    subtraction needed: real scores are O(1)); G *= w; psum accumulation
    of the K identity matmuls; den = reduce(w); out = num/den.
"""

import sys

sys.path.insert(0, "/opt/trn_rl_repo")

import numpy as np
import ml_dtypes

import concourse.bass as bass
import concourse.mybir as mybir
from concourse import bacc, tile
from concourse.bass_utils import run_bass_kernel_spmd

# Problem constants (hardcoded per contest rules).
N_NODES = 50000
CIN = 128
COUT = 64
H = 4
HC = H * COUT  # 256
NCORES = 8
TILE = 128
NLOC = N_NODES // NCORES  # 6250
T = (NLOC + TILE - 1) // TILE  # 49
NEG_SLOPE = 0.2
B_PAD = 300.0

F32 = mybir.dt.float32
FP16 = mybir.dt.float16

YGRP = 7  # tiles per output-write batch


class Plan:
    def __init__(self, K, KOFF, TC, src_ids, node_ids):
        self.K = K  # [T] chunks per tile
        self.KOFF = KOFF  # [T+1] chunk offsets
        self.TC = TC  # total chunks
        self.src_ids = src_ids  # [NCORES][TC*128] int32, -1 = pad slot
        self.node_ids = node_ids  # [NCORES][NLOC] node id per dst position


def _preprocess(edge_index):
    """Integer-only host preprocessing: slot layout per core."""
    src = edge_index[0].astype(np.int64)
    dst = edge_index[1].astype(np.int64)
    E = src.size

    deg = np.bincount(dst, minlength=N_NODES) + 1  # incl self loop
    order = np.argsort(-deg, kind="stable")  # rank -> node
    rank = np.empty(N_NODES, np.int64)
    rank[order] = np.arange(N_NODES)
    deg_sorted = deg[order]

    # K_t = max degree in rank block [t*1024, (t+1)*1024) = first entry.
    K = [int(deg_sorted[t * TILE * NCORES]) for t in range(T)]
    KOFF = np.concatenate([[0], np.cumsum(K)]).astype(np.int64)
    TC = int(KOFF[-1])
    NSLOT = TC * 128

    r = rank[dst]
    core = r % NCORES
    pos = r // NCORES
    tl = pos // TILE
    sl = pos % TILE

    # k-index: 1 + occurrence count among this dst's edges (self = 0).
    ordidx = np.argsort(r, kind="stable")
    sorted_r = r[ordidx]
    first = np.searchsorted(sorted_r, sorted_r, side="left")
    kk = np.empty(E, np.int64)
    kk[ordidx] = np.arange(E) - first + 1

    col = (KOFF[tl] + kk) * 128 + sl

    src_ids = np.full((NCORES, NSLOT), -1, np.int32)
    node_ids = []
    for c in range(NCORES):
        m = core == c
        src_ids[c][col[m]] = src[m]
        nid = order[c::NCORES]
        node_ids.append(nid)
        # self loops
        p = rank[nid] // NCORES
        cs = KOFF[p // TILE] * 128 + (p % TILE)
        src_ids[c][cs] = nid
    return Plan(K, KOFF, TC, src_ids, node_ids)


def _build_program(plan):
    nc = bacc.Bacc(None, target_bir_lowering=False)
    K = plan.K
    KOFF = plan.KOFF
    TC = plan.TC
    KMAX = max(K)

    xE_in = nc.dram_tensor("xE", [128, TC * 128], FP16, kind="ExternalInput")
    wp_in = nc.dram_tensor("Wp", [128, HC + 8], FP16, kind="ExternalInput")
    id_in = nc.dram_tensor("ident", [128, 128], FP16, kind="ExternalInput")
    y_out = nc.dram_tensor("y", [T * 128, HC], F32, kind="ExternalOutput")

    with tile.TileContext(nc) as tc:
        with (
            tc.tile_pool(name="persist", bufs=1) as pp,
            tc.tile_pool(name="xe", bufs=4) as xe,
            tc.tile_pool(name="gg", bufs=3) as gg,
            tc.tile_pool(name="wx", bufs=3) as wx,
            tc.tile_pool(name="egp", bufs=3) as egp,
            tc.tile_pool(name="psA", bufs=5, space="PSUM") as psA,
            tc.tile_pool(name="pso", bufs=3, space="PSUM") as pso,
            tc.tile_pool(name="fin", bufs=2) as fin,
            tc.tile_pool(name="mk", bufs=4) as mk,
        ):
            wp_sb = pp.tile([128, HC + 8], FP16)
            nc.sync.dma_start(wp_sb[:], wp_in[:])
            id_sb = pp.tile([128, 128], FP16)
            nc.sync.dma_start(id_sb[:], id_in[:])
            sdst = pp.tile([128, T, 4], FP16)

            y_v = y_out.rearrange("(t p) f -> p t f", p=128)
            state = {"yow": None, "y0": 0, "ny": 0}

            def _back(t, Kt, Gt):
                eg = egp.tile([128, KMAX, 4], FP16, tag="eg")
                nc.vector.tensor_add(
                    eg[:, 0:Kt, :],
                    Gt[:, 0:Kt, HC : HC + 4],
                    sdst[:, t, :].unsqueeze(1).broadcast_to([128, Kt, 4]),
                )
                # leaky_relu: (e*0.2) max e
                nc.vector.scalar_tensor_tensor(
                    eg[:, 0:Kt, :],
                    eg[:, 0:Kt, :],
                    NEG_SLOPE,
                    eg[:, 0:Kt, :],
                    mybir.AluOpType.mult,
                    mybir.AluOpType.max,
                )
                w = wx.tile([128, KMAX, 4], FP16, tag="w")
                nc.scalar.activation(
                    w[:, 0:Kt, :],
                    eg[:, 0:Kt, :],
                    mybir.ActivationFunctionType.Exp,
                )
                # xh columns are (o, h)-interleaved so the per-head weight
                # broadcast has a contiguous fp16 inner dim (DVE 2x mode).
                nc.vector.tensor_mul(
                    Gt[:, 0:Kt, 0:HC].rearrange("p c (o h) -> p c o h", h=H),
                    Gt[:, 0:Kt, 0:HC].rearrange("p c (o h) -> p c o h", h=H),
                    w[:, 0:Kt, :]
                    .unsqueeze(2)
                    .broadcast_to([128, Kt, COUT, H]),
                )
                po = pso.tile([128, HC], F32, tag="po")
                for k in range(Kt):
                    nc.tensor.matmul(
                        po[:],
                        id_sb[:],
                        Gt[:, k, 0:HC],
                        start=(k == 0),
                        stop=(k == Kt - 1),
                        skip_group_check=True,
                    )
                dn = mk.tile([128, 4], F32, tag="dn")
                nc.vector.tensor_reduce(
                    dn.unsqueeze(-1),
                    w[:, 0:Kt, :].rearrange("p k h -> p h k"),
                    mybir.AxisListType.X,
                    mybir.AluOpType.add,
                )
                rec = mk.tile([128, 4], F32, tag="rec")
                nc.vector.tensor_scalar(
                    dn[:], dn[:], 1e-30, None, mybir.AluOpType.add
                )
                nc.vector.reciprocal(rec[:], dn[:])
                if state["ny"] == 0:
                    state["yow"] = fin.tile(
                        [128, YGRP, HC], F32, tag="yow", name="yow"
                    )
                    state["y0"] = t
                nc.vector.tensor_mul(
                    state["yow"][:, state["ny"], :].rearrange(
                        "p (o h) -> p o h", h=H
                    ),
                    po.rearrange("p (o h) -> p o h", h=H),
                    rec.unsqueeze(1).broadcast_to([128, COUT, H]),
                )
                state["ny"] += 1
                if state["ny"] == YGRP or t == T - 1:
                    nc.sync.dma_start(
                        y_v[:, state["y0"] : state["y0"] + state["ny"], :],
                        state["yow"][:, 0 : state["ny"], :],
                    )
                    state["ny"] = 0

            pend = []
            for t in range(T):
                Kt = K[t]
                xet = xe.tile([128, KMAX * 128], FP16, tag="xet")
                nc.sync.dma_start(
                    xet[:, 0 : Kt * 128],
                    xE_in[:, KOFF[t] * 128 : (KOFF[t] + Kt) * 128],
                )
                Gt = gg.tile([128, KMAX, HC + 4], FP16, tag="G")
                for k in range(Kt):
                    ps = psA.tile([128, HC + 8], F32, tag="psA")
                    nc.tensor.matmul(
                        ps[:],
                        xet[:, k * 128 : (k + 1) * 128],
                        wp_sb[:],
                        start=True,
                        stop=True,
                    )
                    # Drain split ~30/70 DVE/ACT to balance engine load
                    # (DVE also owns the alpha-multiply).
                    if k % 7 < 2:
                        nc.vector.tensor_copy(
                            Gt[:, k, 0 : HC + 4], ps[:, 0 : HC + 4]
                        )
                    else:
                        nc.scalar.copy(Gt[:, k, 0 : HC + 4], ps[:, 0 : HC + 4])
                    if k == 0:
                        nc.vector.tensor_copy(
                            sdst[:, t, :], ps[:, HC + 4 : HC + 8]
                        )
                pend.append((t, Kt, Gt))
                if len(pend) > 2:
                    _back(*pend.pop(0))
            while pend:
                _back(*pend.pop(0))

    nc.compile()
    return nc


def _make_in_maps(plan, x, W, a):
    # Wp = [W^T (256, (o,h)-interleaved) | W^T a_j (4) | W^T a_i (4)].
    Wt = np.ascontiguousarray(W.transpose(2, 1, 0).reshape(CIN, HC))
    a_i = a[:, :COUT]  # dst
    a_j = a[:, COUT:]  # src
    w_aj = np.einsum("hoc,ho->ch", W, a_j)  # [cin, H]
    w_ai = np.einsum("hoc,ho->ch", W, a_i)
    Wp = np.concatenate([Wt, w_aj, w_ai], axis=1).astype(np.float16)

    # x_pad: scores -B for all heads on both src and dst sides.
    A = np.concatenate([w_aj.T, w_ai.T], axis=0)  # [8, cin]
    xpad = np.linalg.lstsq(A, -B_PAD * np.ones(8), rcond=None)[0]

    XT = np.ascontiguousarray(x.T).astype(np.float16)  # [128, N]
    xpad16 = xpad.astype(np.float16)

    ident = np.eye(128, dtype=np.float16)

    in_maps = []
    for c in range(NCORES):
        ids = plan.src_ids[c]
        ids2 = np.where(ids >= 0, ids, 0)
        xe = XT[:, ids2]
        xe[:, ids < 0] = xpad16[:, None]
        in_maps.append(
            {
                "xE": np.ascontiguousarray(xe),
                "Wp": Wp,
                "ident": ident,
            }
        )
    return in_maps


_CACHE = {}


def _get_compiled(edge_key, edge_index):
    if edge_key not in _CACHE:
        plan = _preprocess(edge_index)
        nc = _build_program(plan)
        _CACHE[edge_key] = (plan, nc)
    return _CACHE[edge_key]


def kernel(x, edge_index, W, a, num_nodes, _trace=False):
    x = np.asarray(x)
    edge_index = np.asarray(edge_index)
    W = np.asarray(W)
    a = np.asarray(a)

    edge_key = hash(edge_index.tobytes())
    plan, nc = _get_compiled(edge_key, edge_index)
    in_maps = _make_in_maps(plan, x, W, a)

    kw = {}
    if _trace:
        kw = dict(trace=True)
    res = run_bass_kernel_spmd(nc, in_maps, core_ids=list(range(NCORES)), **kw)

    out = np.empty((N_NODES, HC), np.float32)
    for c in range(NCORES):
        y = res.results[c]["y"][:NLOC]
        # device columns are (o, h); reference wants (h, o)
        out[plan.node_ids[c]] = (
            y.reshape(NLOC, COUT, H).transpose(0, 2, 1).reshape(NLOC, HC)
        )
    if _trace:
        return out, res
    return out


# revision 14
# speedup vs baseline: 1.3641x; 1.1764x over previous
"""Multi-head GAT layer on 8 Trainium2 NeuronCores.

Edge-major streaming design (zero gather):
  - Nodes are ranked by in-degree (desc) and dealt round-robin to the 8
    cores, so every core's destination tile t draws from the same global
    rank block and shares one max-degree K_t -> one SPMD program, zero
    cross-core chunk padding.
  - Destination slot = partition; chunk k of tile t holds the k-th
    in-neighbor of each of the tile's 128 dsts (chunk 0 = self loop).
    The scatter matrix is therefore the IDENTITY for every chunk and the
    per-edge s_dst needs no gather: it is sdst[tile] at the same slot.
  - The host streams x columns in edge order (duplicated per edge) as
    fp16.  The device projects each 128-edge chunk with ONE matmul
    against Wp = [W^T | W^T a_j | W^T a_i] (264 cols), so each psum row
    is [xh(256) | s_src(4) | s_dst(4)].  No DRAM feature table, no
    dma_gather (descriptor generation on GPSIMD was the baseline's
    bottleneck: 939us), no one-hot P/PT streams.
  - Pad slots get a crafted x_pad column solving
    [W^T a_j | W^T a_i]^T x = -300, so their scores are ~-600, their
    exp-weight flushes to exactly 0 in fp16, and they contribute nothing
    to numerator or denominator.
  - Per tile: eg = s_dst + s_src; leaky_relu; w = exp(eg) (no max
    subtraction needed: real scores are O(1)); G *= w; psum accumulation
    of the K identity matmuls; den = reduce(w); out = num/den.
"""

import sys

sys.path.insert(0, "/opt/trn_rl_repo")

import numpy as np
import ml_dtypes

import concourse.bass as bass
import concourse.mybir as mybir
from concourse import bacc, tile
from concourse.bass_utils import run_bass_kernel_spmd

# Problem constants (hardcoded per contest rules).
N_NODES = 50000
CIN = 128
COUT = 64
H = 4
HC = H * COUT  # 256
NCORES = 8
TILE = 128
NLOC = N_NODES // NCORES  # 6250
T = (NLOC + TILE - 1) // TILE  # 49
NEG_SLOPE = 0.2
B_PAD = 300.0

F32 = mybir.dt.float32
FP16 = mybir.dt.float16

YGRP = 7  # tiles per output-write batch


class Plan:
    def __init__(self, K, KOFF, TC, src_ids, node_ids):
        self.K = K  # [T] chunks per tile
        self.KOFF = KOFF  # [T+1] chunk offsets
        self.TC = TC  # total chunks
        self.src_ids = src_ids  # [NCORES][TC*128] int32, -1 = pad slot
        self.node_ids = node_ids  # [NCORES][NLOC] node id per dst position


def _preprocess(edge_index):
    """Integer-only host preprocessing: slot layout per core."""
    src = edge_index[0].astype(np.int64)
    dst = edge_index[1].astype(np.int64)
    E = src.size

    deg = np.bincount(dst, minlength=N_NODES) + 1  # incl self loop
    order = np.argsort(-deg, kind="stable")  # rank -> node
    rank = np.empty(N_NODES, np.int64)
    rank[order] = np.arange(N_NODES)
    deg_sorted = deg[order]

    # K_t = max degree in rank block [t*1024, (t+1)*1024) = first entry.
    K = [int(deg_sorted[t * TILE * NCORES]) for t in range(T)]
    KOFF = np.concatenate([[0], np.cumsum(K)]).astype(np.int64)
    TC = int(KOFF[-1])
    NSLOT = TC * 128

    r = rank[dst]
    core = r % NCORES
    pos = r // NCORES
    tl = pos // TILE
    sl = pos % TILE

    # k-index: 1 + occurrence count among this dst's edges (self = 0).
    ordidx = np.argsort(r, kind="stable")
    sorted_r = r[ordidx]
    first = np.searchsorted(sorted_r, sorted_r, side="left")
    kk = np.empty(E, np.int64)
    kk[ordidx] = np.arange(E) - first + 1

    col = (KOFF[tl] + kk) * 128 + sl

    src_ids = np.full((NCORES, NSLOT), -1, np.int32)
    node_ids = []
    for c in range(NCORES):
        m = core == c
        src_ids[c][col[m]] = src[m]
        nid = order[c::NCORES]
        node_ids.append(nid)
        # self loops
        p = rank[nid] // NCORES
        cs = KOFF[p // TILE] * 128 + (p % TILE)
        src_ids[c][cs] = nid
    return Plan(K, KOFF, TC, src_ids, node_ids)


def _build_program(plan):
    nc = bacc.Bacc(None, target_bir_lowering=False)
    K = plan.K
    KOFF = plan.KOFF
    TC = plan.TC
    KMAX = max(K)

    xE_in = nc.dram_tensor("xE", [128, TC * 128], FP16, kind="ExternalInput")
    wp_in = nc.dram_tensor("Wp", [128, HC + 8], FP16, kind="ExternalInput")
    id_in = nc.dram_tensor("ident", [128, 128], FP16, kind="ExternalInput")
    y_out = nc.dram_tensor("y", [T * 128, HC], F32, kind="ExternalOutput")

    with tile.TileContext(nc) as tc:
        with (
            tc.tile_pool(name="persist", bufs=1) as pp,
            tc.tile_pool(name="xe", bufs=4) as xe,
            tc.tile_pool(name="gg", bufs=3) as gg,
            tc.tile_pool(name="wx", bufs=3) as wx,
            tc.tile_pool(name="egp", bufs=3) as egp,
            tc.tile_pool(name="psA", bufs=3, space="PSUM") as psA,
            tc.tile_pool(name="pso", bufs=2, space="PSUM") as pso,
            tc.tile_pool(name="fin", bufs=2) as fin,
            tc.tile_pool(name="mk", bufs=4) as mk,
        ):
            wp_sb = pp.tile([128, HC + 8], FP16)
            nc.sync.dma_start(wp_sb[:], wp_in[:])
            id_sb = pp.tile([128, 128], FP16)
            nc.sync.dma_start(id_sb[:], id_in[:])
            sdst = pp.tile([128, T, 4], FP16)

            y_v = y_out.rearrange("(t p) f -> p t f", p=128)
            state = {"yow": None, "y0": 0, "ny": 0}

            def _back(t, Kt, Gt):
                eg = egp.tile([128, KMAX, 4], FP16, tag="eg")
                nc.vector.tensor_add(
                    eg[:, 0:Kt, :],
                    Gt[:, 0:Kt, HC : HC + 4],
                    sdst[:, t, :].unsqueeze(1).broadcast_to([128, Kt, 4]),
                )
                # leaky_relu: (e*0.2) max e
                nc.vector.scalar_tensor_tensor(
                    eg[:, 0:Kt, :],
                    eg[:, 0:Kt, :],
                    NEG_SLOPE,
                    eg[:, 0:Kt, :],
                    mybir.AluOpType.mult,
                    mybir.AluOpType.max,
                )
                w = wx.tile([128, KMAX, 4], FP16, tag="w")
                nc.scalar.activation(
                    w[:, 0:Kt, :],
                    eg[:, 0:Kt, :],
                    mybir.ActivationFunctionType.Exp,
                )
                # xh columns are (o, h)-interleaved so the per-head weight
                # broadcast has a contiguous fp16 inner dim (DVE 2x mode).
                nc.vector.tensor_mul(
                    Gt[:, 0:Kt, 0:HC].rearrange("p c (o h) -> p c o h", h=H),
                    Gt[:, 0:Kt, 0:HC].rearrange("p c (o h) -> p c o h", h=H),
                    w[:, 0:Kt, :]
                    .unsqueeze(2)
                    .broadcast_to([128, Kt, COUT, H]),
                )
                po = pso.tile([128, HC], F32, tag="po")
                for k in range(Kt):
                    nc.tensor.matmul(
                        po[:],
                        id_sb[:],
                        Gt[:, k, 0:HC],
                        start=(k == 0),
                        stop=(k == Kt - 1),
                        skip_group_check=True,
                    )
                dn = mk.tile([128, 4], F32, tag="dn")
                nc.vector.tensor_reduce(
                    dn.unsqueeze(-1),
                    w[:, 0:Kt, :].rearrange("p k h -> p h k"),
                    mybir.AxisListType.X,
                    mybir.AluOpType.add,
                )
                rec = mk.tile([128, 4], F32, tag="rec")
                nc.vector.tensor_scalar(
                    dn[:], dn[:], 1e-30, None, mybir.AluOpType.add
                )
                nc.vector.reciprocal(rec[:], dn[:])
                if state["ny"] == 0:
                    state["yow"] = fin.tile(
                        [128, YGRP, HC], F32, tag="yow", name="yow"
                    )
                    state["y0"] = t
                nc.vector.tensor_mul(
                    state["yow"][:, state["ny"], :].rearrange(
                        "p (o h) -> p o h", h=H
                    ),
                    po.rearrange("p (o h) -> p o h", h=H),
                    rec.unsqueeze(1).broadcast_to([128, COUT, H]),
                )
                state["ny"] += 1
                if state["ny"] == YGRP or t == T - 1:
                    nc.sync.dma_start(
                        y_v[:, state["y0"] : state["y0"] + state["ny"], :],
                        state["yow"][:, 0 : state["ny"], :],
                    )
                    state["ny"] = 0

            pend = []
            for t in range(T):
                Kt = K[t]
                xet = xe.tile([128, KMAX * 128], FP16, tag="xet")
                nc.sync.dma_start(
                    xet[:, 0 : Kt * 128],
                    xE_in[:, KOFF[t] * 128 : (KOFF[t] + Kt) * 128],
                )
                Gt = gg.tile([128, KMAX, HC + 4], FP16, tag="G")
                # Two chunks share one 2-bank psum tile (each chunk bank-
                # aligned at a 512-f32 slot) so one strided copy drains both,
                # halving the per-instruction PSUM-access latency burn.
                ps = None
                for k in range(Kt):
                    j = k % 2
                    if j == 0:
                        ps = psA.tile([128, 2, 512], F32, tag="psA")
                    nc.tensor.matmul(
                        ps[:, j, 0 : HC + 8],
                        xet[:, k * 128 : (k + 1) * 128],
                        wp_sb[:],
                        start=True,
                        stop=True,
                        skip_group_check=True,
                    )
                    if k == 0:
                        nc.vector.tensor_copy(
                            sdst[:, t, :], ps[:, 0, HC + 4 : HC + 8]
                        )
                    if j == 1 or k == Kt - 1:
                        nj = j + 1
                        k0 = k - j
                        # ~30/70 DVE/ACT drain split (DVE also owns the
                        # alpha-multiply).
                        if (k0 // 2) % 7 < 2:
                            nc.vector.tensor_copy(
                                Gt[:, k0 : k0 + nj, 0 : HC + 4],
                                ps[:, 0:nj, 0 : HC + 4],
                            )
                        else:
                            nc.scalar.copy(
                                Gt[:, k0 : k0 + nj, 0 : HC + 4],
                                ps[:, 0:nj, 0 : HC + 4],
                            )
                pend.append((t, Kt, Gt))
                if len(pend) > 2:
                    _back(*pend.pop(0))
            while pend:
                _back(*pend.pop(0))

    nc.compile()
    return nc


def _make_in_maps(plan, x, W, a):
    # Wp = [W^T (256, (o,h)-interleaved) | W^T a_j (4) | W^T a_i (4)].
    Wt = np.ascontiguousarray(W.transpose(2, 1, 0).reshape(CIN, HC))
    a_i = a[:, :COUT]  # dst
    a_j = a[:, COUT:]  # src
    w_aj = np.einsum("hoc,ho->ch", W, a_j)  # [cin, H]
    w_ai = np.einsum("hoc,ho->ch", W, a_i)
    Wp = np.concatenate([Wt, w_aj, w_ai], axis=1).astype(np.float16)

    # x_pad: scores -B for all heads on both src and dst sides.
    A = np.concatenate([w_aj.T, w_ai.T], axis=0)  # [8, cin]
    xpad = np.linalg.lstsq(A, -B_PAD * np.ones(8), rcond=None)[0]

    XT = np.ascontiguousarray(x.T).astype(np.float16)  # [128, N]
    xpad16 = xpad.astype(np.float16)

    ident = np.eye(128, dtype=np.float16)

    in_maps = []
    for c in range(NCORES):
        ids = plan.src_ids[c]
        ids2 = np.where(ids >= 0, ids, 0)
        xe = XT[:, ids2]
        xe[:, ids < 0] = xpad16[:, None]
        in_maps.append(
            {
                "xE": np.ascontiguousarray(xe),
                "Wp": Wp,
                "ident": ident,
            }
        )
    return in_maps


_CACHE = {}


def _get_compiled(edge_key, edge_index):
    if edge_key not in _CACHE:
        plan = _preprocess(edge_index)
        nc = _build_program(plan)
        _CACHE[edge_key] = (plan, nc)
    return _CACHE[edge_key]


def kernel(x, edge_index, W, a, num_nodes, _trace=False):
    x = np.asarray(x)
    edge_index = np.asarray(edge_index)
    W = np.asarray(W)
    a = np.asarray(a)

    edge_key = hash(edge_index.tobytes())
    plan, nc = _get_compiled(edge_key, edge_index)
    in_maps = _make_in_maps(plan, x, W, a)

    kw = {}
    if _trace:
        kw = dict(trace=True)
    res = run_bass_kernel_spmd(nc, in_maps, core_ids=list(range(NCORES)), **kw)

    out = np.empty((N_NODES, HC), np.float32)
    for c in range(NCORES):
        y = res.results[c]["y"][:NLOC]
        # device columns are (o, h); reference wants (h, o)
        out[plan.node_ids[c]] = (
            y.reshape(NLOC, COUT, H).transpose(0, 2, 1).reshape(NLOC, HC)
        )
    if _trace:
        return out, res
    return out
